# revision 5
# baseline (speedup 1.0000x reference)
"""Multi-head attention (B=4, S=2048, D=768, H=12, d=64) on 8 trn2 NeuronCores.

Sharding: core c handles batch b = c//2 and head-group g = c%2 (6 heads each,
3 head pairs). Column-parallel QKV projections, full attention for its 6
heads, row-parallel output projection; the two partial outputs per batch are
reduced on the host (+ bo and the exact bv @ wo correction).

v2 design (ACT-exp is the roofline at ~197us):
- Projections run in bf16 (x and w DMA'd as bf16), accumulating fp32 in PSUM.
- Q is quantized to fp8e4 (duplicated in 2 slots), K is split into an exact
  (hi, lo) fp8e4 pair.  Scores then run as fp8 DoubleRow matmuls at 0.5
  cycles/row: (k_hi + k_lo)^T q_hat per head, so the only precision loss on
  the scores path is the single e4m3 rounding of q (~1.1% on alpha).
- exp runs on ACT from the [128,1024] score PSUM (even|odd head halves) into
  fp16 e tiles; ctx matmuls are fp16 at 1 cycle/row.
- The V stationary tiles carry an appended ones column, so the ctx PSUM
  accumulates the softmax denominator in row 64 for free (no DVE/GPSIMD
  reduction chains at all).
- Normalize (reciprocal + PE broadcast + DVE multiply) and the bf16 output
  projection are software-pipelined into the next chunk's sk loop; V/Q/K
  projection units are interleaved into early attention chunks so the first
  exp fires ~14us in and ACT stays saturated.
"""
import sys

for _p in ("/opt/trn_rl_repo", "/root/.axon_site/_ro/trn_rl_repo"):
    if _p not in sys.path:
        sys.path.append(_p)

import numpy as np

import concourse.bass as bass  # noqa: F401
import concourse.bacc as bacc
import concourse.mybir as mybir
import concourse.tile as tile
from concourse.bass_utils import run_bass_kernel_spmd

B, S, D = 4, 2048, 768
NUM_HEADS, HEAD = 12, 64
NCORES = 8
HPC = NUM_HEADS // 2          # 6 heads per core
MC = HPC * HEAD               # 384 per-core projection cols
KT = D // 128                 # 6 contraction k-tiles
MT = MC // 128                # 3 head-pair tiles
ST = S // 128                 # 16 key-seq tiles
CW = 512                      # q chunk width
NCH = S // CW                 # 4 q chunks
XC = 512                      # x DMA column chunk
NXC = S // XC                 # 4 x chunks per input

F32 = mybir.dt.float32
F32R = mybir.dt.float32r
BF16 = mybir.dt.bfloat16
F16 = mybir.dt.float16
F8 = mybir.dt.float8e4
DR = mybir.MatmulPerfMode.DoubleRow
EXP = mybir.ActivationFunctionType.Exp
ADD = mybir.AluOpType.add
SUB = mybir.AluOpType.subtract
MULT = mybir.AluOpType.mult

_NC = None
LAST_RESULTS = None
_LAST_IN_MAPS = None


def _build(loop=None):
    nc = bacc.Bacc("TRN2", target_bir_lowering=False, debug=False,
                   num_devices=NCORES)
    xqt = nc.declare_dram_parameter("xqt", [D, S], BF16, isOutput=False)
    xkt = nc.declare_dram_parameter("xkt", [D, S], F8, isOutput=False)
    xvt = nc.declare_dram_parameter("xvt", [D, S], BF16, isOutput=False)
    wq = nc.declare_dram_parameter("wq", [D, MC], BF16, isOutput=False)
    wk = nc.declare_dram_parameter("wk", [D, MC], F8, isOutput=False)
    wv = nc.declare_dram_parameter("wv", [D, MC], BF16, isOutput=False)
    wo = nc.declare_dram_parameter("wo", [MC, D], BF16, isOutput=False)
    bqk = nc.declare_dram_parameter("bqk", [128, 2 * MT], F32, isOutput=False)
    cst_d = nc.declare_dram_parameter("cst", [1, 192], F32R, isOutput=False)
    idn_d = nc.declare_dram_parameter("idn", [128, 128], BF16, isOutput=False)
    out = nc.declare_dram_parameter("out", [S, D], BF16, isOutput=True)

    with tile.TileContext(nc) as tc:
        if loop:
            with tc.For_i(0, loop, 1):
                _emit(nc, tc, xqt, xkt, xvt, wq, wk, wv, wo, bqk, cst_d, idn_d, out)
        else:
            _emit(nc, tc, xqt, xkt, xvt, wq, wk, wv, wo, bqk, cst_d, idn_d, out)
    nc.compile()
    return nc


def _emit(nc, tc, xqt, xkt, xvt, wq, wk, wv, wo, bqk, cst_d, idn_d, out):
    ctx_lp = nc.allow_low_precision(reason="bf16/fp16/fp8 tiles feed the PE; accumulation stays fp32 in PSUM")
    ctx_lp.__enter__()
    with (
        tc.tile_pool(name="cstp", bufs=1) as cst_pool,
        tc.tile_pool(name="wp", bufs=1) as w_pool,
        tc.tile_pool(name="xpq", bufs=NXC) as xq_pool,
        tc.tile_pool(name="xpk", bufs=NXC) as xk_pool,
        tc.tile_pool(name="xpv", bufs=NXC) as xv_pool,
        tc.tile_pool(name="qtp", bufs=1) as qt_pool,
        tc.tile_pool(name="ktp", bufs=1) as kt_pool,
        tc.tile_pool(name="vp", bufs=ST) as v_pool,
        tc.tile_pool(name="ep", bufs=16) as e_pool,
        tc.tile_pool(name="ctxp", bufs=1) as ctx_pool,
        tc.tile_pool(name="rp", bufs=4) as r_pool,
        tc.tile_pool(name="bp", bufs=2) as b_pool,
        tc.tile_pool(name="outp", bufs=4) as out_pool,
        tc.tile_pool(name="psS", bufs=2, space="PSUM") as psS,
        tc.tile_pool(name="psX", bufs=4, space="PSUM") as psX,
    ):
        cst = cst_pool.tile([1, 192], F32R, tag="cst")
        idn = cst_pool.tile([128, 128], BF16, tag="idn")
        bqk_sb = cst_pool.tile([128, 2 * MT], F32, tag="bqk")
        bq_sb = bqk_sb[:, 0:MT]
        bk_sb = bqk_sb[:, MT:2 * MT]
        w_sb = {}
        for name, w, dt_ in (("wq", wq, BF16), ("wk", wk, F8),
                             ("wv", wv, BF16)):
            w_sb[name] = w_pool.tile([128, KT, MC], dt_, tag=name,
                                     name=f"w_{name}")
        wo_sb = w_pool.tile([128, MT, D], BF16, tag="wo")
        x_sb = {}
        for name, dt_, pool in (("xq", BF16, xq_pool), ("xk", F8, xk_pool),
                                ("xv", BF16, xv_pool)):
            x_sb[name] = [pool.tile([128, KT, XC], dt_, tag="xt",
                                    name=f"{name}{c}") for c in range(NXC)]
        # qt: [d-in-pair partition, hp, dup slot, seq] fp8 (both slots = q_hat)
        # kt: same shape, slots = (k_hi, k_lo) with k_hi + k_lo == K exactly
        qt = qt_pool.tile([128, MT, 2, S], F8, tag="qt")
        kt = kt_pool.tile([128, MT, 2, S], F8, tag="kt")
        # vt: [key-seq partition, hp, head(even/odd), 64 V cols + ones col]
        vt = [v_pool.tile([128, MT, 2, 66], F16, tag="v", name=f"vt{st}")
              for st in range(ST)]
        ctx_sb = ctx_pool.tile([128, MT, S], BF16, tag="ctx")
        # bf16 staging for the last q-chunk's partial output projection
        # (m0/m1 passes run early; only the m2 pass remains at the tail)
        acc_sb = ctx_pool.tile([128, NCH, D], BF16, tag="acc")

        # ---------------- DMAs (order = DMA engine schedule) --------------
        def dma_x(name, dram, c, w0=0, w1=XC):
            cols = slice(c * XC + w0, c * XC + w1)
            nc.sync.dma_start(
                out=x_sb[name][c][:, :, w0:w1],
                in_=dram[:, cols].rearrange("(n k) m -> k n m", k=128))

        # DMA order is the critical path to the first exp: only the m0
        # slices of wq/wk plus the first x chunks gate it, so they go first.
        def dma_w(name, dram, m0, m1):
            nc.sync.dma_start(
                out=w_sb[name][:, :, m0 * 128:m1 * 128],
                in_=dram[:, m0 * 128:m1 * 128].rearrange(
                    "(n k) m -> k n m", k=128))

        dma_w("wk", wk, 0, 1)
        dma_x("xk", xkt, 0, 0, 256)
        nc.sync.dma_start(out=bqk_sb, in_=bqk[:])
        dma_w("wq", wq, 0, 1)
        dma_x("xq", xqt, 0)
        nc.sync.dma_start(out=cst, in_=cst_d[:])
        dma_x("xk", xkt, 0, 256, XC)
        dma_x("xk", xkt, 1)
        dma_x("xk", xkt, 2)
        dma_x("xk", xkt, 3)
        nc.sync.dma_start(out=w_sb["wv"],
                          in_=wv[:].rearrange("(n k) m -> k n m", k=128))
        dma_x("xv", xvt, 0)
        dma_x("xq", xqt, 1)
        dma_x("xv", xvt, 1)
        dma_w("wq", wq, 1, 3)
        dma_w("wk", wk, 1, 3)
        dma_x("xv", xvt, 2)
        dma_x("xv", xvt, 3)
        dma_x("xq", xqt, 2)
        dma_x("xq", xqt, 3)
        nc.sync.dma_start(out=wo_sb,
                          in_=wo[:].rearrange("(t p) o -> p t o", p=128))
        nc.sync.dma_start(out=idn, in_=idn_d[:])

        for st in range(ST):
            nc.gpsimd.memset(vt[st][:, :, :, 64:65], 1.0)

        # ---------------- projection emission units -----------------------
        def qk_unit(wname, m, c, w0=0, w1=XC):
            def emit():
                ps = psX.tile([128, CW], F32, tag="ps",
                              name=f"ps_{wname}{m}{c}{w0}")[:, 0:w1 - w0]
                if wname == "wk":
                    for k in range(0, KT, 2):
                        nc.tensor.matmul(
                            ps, w_sb["wk"][:, k:k + 2, m * 128:(m + 1) * 128],
                            x_sb["xk"][c][:, k:k + 2, w0:w1],
                            perf_mode=DR,
                            start=(k == 0), stop=(k == KT - 2),
                            skip_group_check=True)
                else:
                    for k in range(KT):
                        nc.tensor.matmul(
                            ps, w_sb["wq"][:, k, m * 128:(m + 1) * 128],
                            x_sb["xq"][c][:, k, w0:w1],
                            start=(k == 0), stop=(k == KT - 1),
                            skip_group_check=True)
                sl = slice(c * XC + w0, c * XC + w1)
                if wname == "wq":
                    nc.vector.tensor_scalar_add(qt[:, m, 0, sl], ps,
                                                bq_sb[:, m:m + 1])
                    nc.vector.tensor_scalar_add(qt[:, m, 1, sl], ps,
                                                bq_sb[:, m:m + 1])
                else:
                    nc.vector.tensor_scalar_add(kt[:, m, 0, sl], ps,
                                                bk_sb[:, m:m + 1])
                    nc.vector.scalar_tensor_tensor(
                        kt[:, m, 1, sl], ps, bk_sb[:, m:m + 1],
                        kt[:, m, 0, sl], op0=ADD, op1=SUB)
            return emit

        def v_unit(st):
            def emit():
                c, st8 = st // 4, st % 4
                psf = psX.tile([128, CW], F32, tag="ps", name=f"psV{st}")
                ps = psf[:, 0:MC]
                for k in range(KT):
                    nc.tensor.matmul(
                        ps, x_sb["xv"][c][:, k, st8 * 128:(st8 + 1) * 128],
                        w_sb["wv"][:, k, :],
                        start=(k == 0), stop=(k == KT - 1),
                        skip_group_check=True)
                psv = ps.rearrange("p (t two d) -> p t two d", two=2, d=HEAD)
                nc.vector.tensor_copy(vt[st][:, :, :, 0:HEAD], psv)
            return emit

        # Q-m0-c0/K-m0-c0 upfront: chunk (hp0, sc0) needs q cols 0:512 and
        # the k-seq blocks as they stream in.  Everything else goes on a
        # global work queue popped one unit per sk slot: K-m0-c1 first
        # (needed by sc0's sk8+), then the V tiles in ctx order (ctx
        # emission is gated on v_done), then the remaining projections.
        qk_unit("wk", 0, 0, 0, 256)()
        qk_unit("wq", 0, 0)()
        work = [("k1", ("wk", 0, 9), qk_unit("wk", 0, 0, 256, XC)),
                ("k1", ("wk", 0, 1), qk_unit("wk", 0, 1)),
                ("k1", ("wk", 0, 2), qk_unit("wk", 0, 2)),
                ("k1", ("wk", 0, 3), qk_unit("wk", 0, 3))]
        work += [("v", None, v_unit(st)) for st in range(ST)]
        work += [("p", ("wq", 0, c), qk_unit("wq", 0, c)) for c in (1, 2, 3)]
        for m in (1, 2):
            for wname in ("wq", "wk"):
                for c in range(NXC):
                    work.append(("p", (wname, m, c), qk_unit(wname, m, c)))
        work.reverse()  # pop() from the front order

        def force_prereqs(hp, sc):
            # emission-order safety: any still-queued projection unit this
            # chunk's scores read must be emitted before its first scores
            # matmul.  k1 units are excluded: their in-loop pops (sk 0..3 of
            # chunk 0) always precede the k-blocks that read them.
            need = [ent for ent in work
                    if ent[0] == "p" and ent[1] is not None
                    and ent[1][1] == hp
                    and (ent[1][0] == "wk" or ent[1][2] == sc)]
            for ent in need:
                work.remove(ent)
                ent[2]()

        def pop_unit(types):
            for i in range(len(work) - 1, -1, -1):
                if work[i][0] in types:
                    ent = work.pop(i)
                    ent[2]()
                    return ent
            return None

        # ---------------- attention -------------------------------------
        def emit_ctx(hp, sk, e, ctx_e, ctx_o):
            nc.tensor.matmul(ctx_e, vt[sk][:, hp, 0, 0:65], e[:, 0:CW],
                             start=(sk == 0), stop=(sk == ST - 1),
                             skip_group_check=True)
            nc.tensor.matmul(ctx_o, vt[sk][:, hp, 1, 0:65], e[:, CW:2 * CW],
                             start=(sk == 0), stop=(sk == ST - 1),
                             skip_group_check=True)

        def emit_recips(pend):
            hp, sc, ctx_e, ctx_o = pend
            r_e = r_pool.tile([1, CW], F32R, tag="r", name=f"re{hp}{sc}")
            r_o = r_pool.tile([1, CW], F32R, tag="r", name=f"ro{hp}{sc}")
            nc.vector.reciprocal(r_o, ctx_o[64:65, :])
            nc.vector.reciprocal(r_e, ctx_e[64:65, :])
            return r_e, r_o

        def emit_norm(pend, rr, at_tail=False):
            hp, sc, ctx_e, ctx_o = pend
            r_e, r_o = rr
            ps_b = psX.tile([128, CW], F32, tag="ps", name=f"psB{hp}{sc}")
            nc.tensor.matmul(ps_b, cst[0:1, 64:192], r_o,
                             start=True, stop=False, skip_group_check=True)
            nc.tensor.matmul(ps_b[0:64, :], cst[0:1, 0:64], r_e,
                             start=False, stop=True, skip_group_check=True)
            b_sb = b_pool.tile([128, CW], F32, tag="bsb")
            if at_tail:
                nc.scalar.copy(b_sb, ps_b)
            else:
                nc.vector.tensor_copy(b_sb, ps_b)
            sl = slice(sc * CW, (sc + 1) * CW)
            nc.vector.tensor_tensor(ctx_sb[0:64, hp, sl], ctx_e[0:64, :],
                                    b_sb[0:64, :], op=MULT)
            nc.vector.tensor_tensor(ctx_sb[64:128, hp, sl], ctx_o[0:64, :],
                                    b_sb[64:128, :], op=MULT)

        def op_pass_unit(m, st4):
            def emit():
                s0 = (NCH - 1) * CW + st4 * 128
                for n0, nw in ((0, 512), (512, 256)):
                    psf = psX.tile([128, CW], F32, tag="ps",
                                   name=f"psP{m}{st4}{n0}")[:, 0:nw]
                    nc.tensor.matmul(psf, ctx_sb[:, m, s0:s0 + 128],
                                     wo_sb[:, m, n0:n0 + nw],
                                     start=True, stop=True,
                                     skip_group_check=True)
                    if m == 0:
                        nc.vector.tensor_copy(acc_sb[:, st4, n0:n0 + nw], psf)
                    else:
                        nc.vector.tensor_tensor(acc_sb[:, st4, n0:n0 + nw],
                                                acc_sb[:, st4, n0:n0 + nw],
                                                psf, op=ADD)
            return emit

        def outproj_tail():
            sc = NCH - 1
            o_big = ctx_pool.tile([128, CW // 128, D], BF16, tag="obig")
            for st4 in range(CW // 128):
                s0 = sc * CW + st4 * 128
                for n0, nw in ((0, 512), (512, 256)):
                    psf = psX.tile([128, CW], F32, tag="ps",
                                   name=f"psT{st4}{n0}")[:, 0:nw]
                    nc.tensor.matmul(psf, ctx_sb[:, MT - 1, s0:s0 + 128],
                                     wo_sb[:, MT - 1, n0:n0 + nw],
                                     start=True, stop=False,
                                     skip_group_check=True)
                    nc.tensor.matmul(psf, idn,
                                     acc_sb[:, st4, n0:n0 + nw],
                                     start=False, stop=True,
                                     skip_group_check=True)
                    if st4 % 2:
                        nc.vector.tensor_copy(o_big[:, st4, n0:n0 + nw], psf)
                    else:
                        nc.scalar.copy(o_big[:, st4, n0:n0 + nw], psf)
                if st4 == 1:
                    nc.sync.dma_start(
                        out=out[sc * CW:sc * CW + 256, :].rearrange(
                            "(f p) o -> p f o", p=128),
                        in_=o_big[:, 0:2])
            nc.sync.dma_start(
                out=out[sc * CW + 256:(sc + 1) * CW, :].rearrange(
                    "(f p) o -> p f o", p=128),
                in_=o_big[:, 2:4])

        def outproj_rounds(sc, at_tail=False):
            for st4 in range(CW // 128):
                s0 = sc * CW + st4 * 128
                o_sb = out_pool.tile([128, D], BF16, tag="osb")
                for n0, nw in ((0, 512), (512, 256)):
                    ps_of = psX.tile([128, CW], F32, tag="ps",
                                     name=f"psO{sc}{st4}{n0}")
                    ps_o = ps_of[:, 0:nw]
                    for m in range(MT):
                        nc.tensor.matmul(ps_o, ctx_sb[:, m, s0:s0 + 128],
                                         wo_sb[:, m, n0:n0 + nw],
                                         start=(m == 0), stop=(m == MT - 1),
                                         skip_group_check=True)
                    if at_tail:
                        nc.scalar.copy(o_sb[:, n0:n0 + nw], ps_o)
                    else:
                        nc.vector.tensor_copy(o_sb[:, n0:n0 + nw], ps_o)
                    nc.sync.dma_start(out=out[s0:s0 + 128, n0:n0 + nw],
                                      in_=o_sb[:, n0:n0 + nw])
                    yield

        # Per-sk scheduler.  psX has 4 banks; at most one ctx pair (2
        # banks) is alive (the previous pair is freed by its normalize
        # before the current pair is allocated) plus short-lived transient
        # tiles (V/proj/outproj/broadcast psum).  The previous chunk's
        # unfinished ctx matmuls (backlog) flush 2-per-sk at the start of
        # the next chunk so the PE queue never head-blocks on exp, and all
        # ctx emission is gated on the V tiles it needs being projected.
        pending = None          # dict: prior chunk awaiting ctx-flush+norm
        pending_out = None      # output-projection generator
        v_done = 0              # V tiles projected so far (global)
        for hp in range(MT):
            for sc in range(NCH):
                first = hp == 0 and sc == 0
                force_prereqs(hp, sc)
                ctx_e = ctx_o = None
                sq = slice(sc * CW, (sc + 1) * CW)
                backlog = []
                for sk in range(ST):
                    sks = slice(sk * 128, (sk + 1) * 128)
                    ps = psS.tile([128, 2 * CW], F32, tag="psS")
                    nc.tensor.matmul(ps[:, 0:CW], kt[0:64, hp, :, sks],
                                     qt[0:64, hp, :, sq], perf_mode=DR,
                                     skip_group_check=True)
                    nc.tensor.matmul(ps[:, CW:2 * CW], kt[64:128, hp, :, sks],
                                     qt[64:128, hp, :, sq], perf_mode=DR,
                                     skip_group_check=True)
                    e = e_pool.tile([128, 2 * CW], F16, tag="e")
                    nc.scalar.activation(e, ps, EXP, scale=0.125)
                    backlog.append((sk, e))

                    # pipeline-state chain: flush prev ctx -> recip -> norm
                    # -> alloc own ctx pair (psX-safe ordering)
                    v_left = any(w[0] == "v" for w in work)
                    if pending is not None:
                        p = pending
                        if p["backlog"]:
                            for _ in range(1 if v_left else 2):
                                if p["backlog"] and p["backlog"][0][0] < v_done:
                                    bsk, be = p["backlog"].pop(0)
                                    emit_ctx(p["hp"], bsk, be,
                                             p["ctx_e"], p["ctx_o"])
                        elif "rr" not in p:
                            p["rr"] = emit_recips(
                                (p["hp"], p["sc"], p["ctx_e"], p["ctx_o"]))
                        else:
                            emit_norm((p["hp"], p["sc"], p["ctx_e"],
                                       p["ctx_o"]), p["rr"])
                            if p["sc"] == NCH - 1 and p["hp"] < MT - 1:
                                for st4 in range(CW // 128):
                                    work.insert(0, ("p", None, op_pass_unit(
                                        p["hp"], st4)))
                            if p["hp"] == MT - 1 and p["sc"] < NCH - 1:
                                pending_out = outproj_rounds(p["sc"])
                            pending = None
                    elif ctx_e is None:
                        ctx_ef = psX.tile([128, CW], F32, tag="ps",
                                          name=f"ce{hp}{sc}")
                        ctx_of = psX.tile([128, CW], F32, tag="ps",
                                          name=f"co{hp}{sc}")
                        ctx_e = ctx_ef[0:65, :]
                        ctx_o = ctx_of[0:65, :]

                    # one work-queue / outproj unit per sk slot
                    if first and sk in (0, 1, 2, 3):
                        pop_unit(("k1",))
                    elif first and sk == 12 and any(w[0] == "p" for w in work):
                        pop_unit(("p",))
                    elif (not first or sk >= 7) and pop_unit(("v",)):
                        v_done += 1
                    elif pending_out is not None and sk % 2 == 1:
                        if next(pending_out, StopIteration) is StopIteration:
                            pending_out = None
                    elif not first and sk in (4, 8, 11, 14):
                        pop_unit(("p",))

                    # drain this chunk's ctx backlog (V-gated); while V
                    # projections are still streaming, hold ctx back (the
                    # e pool buffers it) unless pool pressure forces it
                    if ctx_e is not None:
                        if v_left and len(backlog) < 11:
                            quota = 0
                        elif (hp, sc) == (MT - 1, NCH - 1) or len(backlog) > 6:
                            quota = 2
                        else:
                            quota = 1
                        while quota and backlog and backlog[0][0] < v_done:
                            bsk, be = backlog.pop(0)
                            emit_ctx(hp, bsk, be, ctx_e, ctx_o)
                            quota -= 1
                while pending is not None:
                    p = pending
                    if p["backlog"]:
                        if p["backlog"][0][0] >= v_done:
                            assert pop_unit(("v",)) is not None
                            v_done += 1
                        else:
                            bsk, be = p["backlog"].pop(0)
                            emit_ctx(p["hp"], bsk, be, p["ctx_e"], p["ctx_o"])
                    elif "rr" not in p:
                        p["rr"] = emit_recips(
                            (p["hp"], p["sc"], p["ctx_e"], p["ctx_o"]))
                    else:
                        emit_norm((p["hp"], p["sc"], p["ctx_e"],
                                   p["ctx_o"]), p["rr"])
                        if p["sc"] == NCH - 1 and p["hp"] < MT - 1:
                            for st4 in range(CW // 128):
                                work.insert(0, ("p", None, op_pass_unit(
                                    p["hp"], st4)))
                        if p["hp"] == MT - 1 and p["sc"] < NCH - 1:
                            pending_out = outproj_rounds(p["sc"])
                        pending = None
                pending = {"hp": hp, "sc": sc, "ctx_e": ctx_e,
                           "ctx_o": ctx_o, "backlog": backlog}
        # tail: drain leftovers, then last chunk's flush/normalize/outproj
        if pending_out is not None:
            for _ in pending_out:
                pass
        while work:
            work.pop()[2]()
        p = pending
        for bsk, be in p["backlog"]:
            emit_ctx(p["hp"], bsk, be, p["ctx_e"], p["ctx_o"])
        rr = emit_recips((p["hp"], p["sc"], p["ctx_e"], p["ctx_o"]))
        emit_norm((p["hp"], p["sc"], p["ctx_e"], p["ctx_o"]), rr, at_tail=True)
        outproj_tail()


def _cst_host():
    c = np.zeros((1, 192), np.float32)
    c[0, 0:64] = 1.0     # even-head broadcast rows 0:64
    c[0, 128:192] = 1.0  # odd-head broadcast rows 64:128 (cols 64:128 zero)
    return c


def kernel(query, key, value, wq, bq, wk, bk, wv, bv, wo, bo):
    global _NC, LAST_RESULTS, _LAST_IN_MAPS
    if _NC is None:
        _NC = _build()

    import ml_dtypes
    bf16 = ml_dtypes.bfloat16
    f8 = ml_dtypes.float8_e4m3

    def f8c(a):
        return np.ascontiguousarray(np.asarray(a).astype(np.float32).astype(f8))

    def bfc(a):
        return np.ascontiguousarray(np.asarray(a).astype(bf16))

    def f32c(a):
        return np.ascontiguousarray(np.asarray(a, dtype=np.float32))

    query, key, value = map(np.asarray, (query, key, value))
    xt = [{"xqt": bfc(query[b].T), "xkt": f8c(key[b].T),
           "xvt": bfc(value[b].T)} for b in range(B)]
    wslices = []
    for g in range(2):
        cols = slice(g * MC, (g + 1) * MC)
        wslices.append({
            "wq": bfc(np.asarray(wq)[:, cols]),
            "wk": f8c(np.asarray(wk)[:, cols]),
            "wv": bfc(np.asarray(wv)[:, cols]),
            "wo": bfc(np.asarray(wo)[cols, :]),
            "bqk": np.stack([f32c(np.asarray(bq)[cols]).reshape(MT, 128).T,
                             f32c(np.asarray(bk)[cols]).reshape(MT, 128).T],
                            ).transpose(1, 0, 2).reshape(128, 2 * MT).copy(),
            "cst": _cst_host(),
            "idn": np.eye(128).astype(bf16),
        })
    in_maps = [dict(xt[c // 2], **wslices[c % 2]) for c in range(NCORES)]

    _LAST_IN_MAPS = in_maps
    res = run_bass_kernel_spmd(_NC, in_maps, core_ids=list(range(NCORES)))
    LAST_RESULTS = res

    corr = (np.asarray(bv, np.float64) @ np.asarray(wo, np.float64)
            + np.asarray(bo, np.float64)).astype(np.float32)
    y = np.empty((B, S, D), np.float32)
    for b in range(B):
        y[b] = (res.results[2 * b]["out"].astype(np.float32)
                + res.results[2 * b + 1]["out"].astype(np.float32) + corr)
    return y
